# revision 1
# baseline (speedup 1.0000x reference)
"""Trainium2 Bass kernel for nn_BasicDeconvolutionBlock (sparse transposed conv + BN + ReLU).

Self-contained: hardcodes problem shapes; shards across 8 NeuronCores by
output-site owner; runs one SPMD Bass/Tile program via run_bass_kernel_spmd.

Pipeline per core (out rows [75000c, 75000(c+1))):
  phase A: pairs sorted by (k, local_row); per 128-pair chunk:
      indirect-gather feats rows -> PE transpose -> matmul with W[k] -> C (fp16, DRAM)
  phase B: per 128-row window: indirect-gather its C rows (sorted, padded to
      uniform chunk count), build one-hot SelT via is_equal vs IOTA, matmul-
      accumulate window rows in PSUM; per-channel sum/sumsq stats accumulated
      in PSUM across all windows.
  BN: AllReduce [2,96] stats across 8 cores, scale/bias, normalize+ReLU pass.
"""
import os
import sys
import numpy as np

sys.path.insert(0, "/opt/trn_rl_repo")

N_IN = 200000
N_OUT = 600000
K = 27
P = 150000
C = 96
BN_EPS = 1e-5
NCORES = 8
R_CORE = N_OUT // NCORES          # 75000
NWIN = (R_CORE + 127) // 128      # 586
R_PAD = NWIN * 128                # 75008

_EXEC_TIME_NS = [None]


def _host_prep(in_idx, out_idx):
    """Build per-core index/rowid arrays. Returns dict of numpy arrays + constants."""
    kk = np.repeat(np.arange(K, dtype=np.int64), P)          # [K*P]
    src = in_idx.reshape(-1).astype(np.int64)                # feats row per pair
    dst = out_idx.reshape(-1).astype(np.int64)
    owner = dst // R_CORE
    lrow = dst - owner * R_CORE

    # global sort by (owner, k, lrow)
    key = (owner * K + kk) * (R_PAD + 1) + lrow
    order = np.argsort(key, kind="stable")
    src_s = src[order]
    lrow_s = lrow[order]
    group = (owner * K + kk)[order]                          # sorted too

    counts = np.bincount(group, minlength=NCORES * K).reshape(NCORES, K)
    n_k_max = counts.max(axis=0)                             # [K]
    pad_k = ((n_k_max + 127) // 128) * 128                   # per-k padded size
    chunks_k = (pad_k // 128).astype(np.int64)
    S_pad = int(pad_k.sum())
    n_chunks = int(chunks_k.sum())
    k_chunk_base = np.concatenate([[0], np.cumsum(chunks_k)])[:-1]

    g_start = np.concatenate([[0], np.cumsum(counts.reshape(-1))])  # per (c,k)

    # per-core slot arrays
    A_idx = np.full((NCORES, n_chunks * 128), N_IN, dtype=np.int32)  # pad -> zero row
    slot_lrow = np.full((NCORES, n_chunks * 128), -1, dtype=np.int32)
    slot_off = np.concatenate([[0], np.cumsum(pad_k)])[:-1]          # slot base per k
    for c in range(NCORES):
        for k in range(K):
            g = c * K + k
            n = counts[c, k]
            a = g_start[g]
            base = int(slot_off[k])
            A_idx[c, base:base + n] = src_s[a:a + n]
            slot_lrow[c, base:base + n] = lrow_s[a:a + n]

    # phase B: per (core, window) the slots sorted by lrow.
    # slots within each k-group are lrow-sorted; concatenating k-runs per window.
    NWC_counts = np.zeros((NCORES, NWIN), dtype=np.int64)
    for c in range(NCORES):
        valid = slot_lrow[c] >= 0
        w = slot_lrow[c][valid] // 128
        NWC_counts[c] = np.bincount(w, minlength=NWIN)
    M_w = int(NWC_counts.max())
    NWC = (M_w + 127) // 128                                  # chunks per window
    S_w = NWC * 128

    B_idx = np.zeros((NCORES, NWIN * S_w), dtype=np.int32)    # C row ids (pad -> 0)
    B_rowid = np.full((NCORES, NWIN * S_w), -1.0, dtype=np.float16)
    for c in range(NCORES):
        valid = np.nonzero(slot_lrow[c] >= 0)[0]
        rows = slot_lrow[c][valid]
        o2 = np.argsort(rows, kind="stable")
        pos = valid[o2].astype(np.int32)                      # C row per sorted slot
        rows = rows[o2]
        w = rows // 128
        rel = (rows - w * 128).astype(np.float16)
        # place into window-padded layout
        wc = np.concatenate([[0], np.cumsum(np.bincount(w, minlength=NWIN))])
        for win in range(NWIN):
            a, b = wc[win], wc[win + 1]
            B_idx[c, win * S_w: win * S_w + (b - a)] = pos[a:b]
            B_rowid[c, win * S_w: win * S_w + (b - a)] = rel[a:b]

    # device layouts: partition-major [128, cols]
    def pmaj(arr, ncols):
        return np.ascontiguousarray(
            arr.reshape(ncols, 128).T)

    prep = {
        "S_pad": S_pad, "n_chunks": n_chunks, "NWC": NWC,
        "chunks_k": chunks_k, "k_chunk_base": k_chunk_base,
        "A_idx": [pmaj(A_idx[c], n_chunks) for c in range(NCORES)],
        "B_idx": [pmaj(B_idx[c], NWIN * NWC) for c in range(NCORES)],
        "B_rowid": [pmaj(B_rowid[c], NWIN * NWC) for c in range(NCORES)],
    }
    return prep


def _build(prep):
    import concourse.bass as bass
    import concourse.bacc as bacc
    import concourse.mybir as mybir
    import concourse.tile as tile

    n_chunks = prep["n_chunks"]
    NWC = prep["NWC"]
    chunks_k = prep["chunks_k"]
    k_chunk_base = prep["k_chunk_base"]
    S_pad = prep["S_pad"]

    f16 = mybir.dt.float16
    f32 = mybir.dt.float32
    i32 = mybir.dt.int32

    nc = bacc.Bacc("TRN2", target_bir_lowering=False, debug=False,
                   num_devices=NCORES)
    feats = nc.dram_tensor("feats", [N_IN + 1, C], f16, kind="ExternalInput")
    wmat = nc.dram_tensor("wmat", [C, K * C], f16, kind="ExternalInput")
    a_idx = nc.dram_tensor("a_idx", [128, n_chunks], i32, kind="ExternalInput")
    b_idx = nc.dram_tensor("b_idx", [128, NWIN * NWC], i32, kind="ExternalInput")
    b_rowid = nc.dram_tensor("b_rowid", [128, NWIN * NWC], f16, kind="ExternalInput")
    consts = nc.dram_tensor("consts", [128, 384], f16, kind="ExternalInput")
    gb = nc.dram_tensor("gb", [1, 2 * C], f32, kind="ExternalInput")
    y = nc.dram_tensor("y", [R_PAD, C], f32, kind="ExternalOutput")

    cdram = nc.dram_tensor("cdram", [S_pad, C], f16)
    outp = nc.dram_tensor("outp", [R_PAD, C], f32)
    cc_in = nc.dram_tensor("cc_in", [1, 2 * C], f32)
    cc_out = nc.dram_tensor("cc_out", [1, 2 * C], f32, addr_space="Shared")

    CB = 4  # C-write batching (chunks per DMA)

    with tile.TileContext(nc) as tc:
        with (
            tc.tile_pool(name="const", bufs=1) as cp,
            tc.tile_pool(name="sb", bufs=12) as sb,
            tc.tile_pool(name="sb2", bufs=12) as sb2,
            tc.tile_pool(name="sbig", bufs=3) as sbig,
            tc.tile_pool(name="ps_t", bufs=2, space="PSUM") as ps_t,
            tc.tile_pool(name="ps_c", bufs=2, space="PSUM") as ps_c,
            tc.tile_pool(name="ps_w", bufs=2, space="PSUM") as ps_w,
            tc.tile_pool(name="ps_s", bufs=1, space="PSUM") as ps_s,
        ):
            # constants
            w_t = cp.tile([C, K * C], f16)
            nc.sync.dma_start(out=w_t[:], in_=wmat[:])
            cst = cp.tile([128, 384], f16)
            nc.sync.dma_start(out=cst[:], in_=consts[:])
            ident = cst[:, 0:128]          # identity 128x128
            iota = cst[:, 128:256]         # IOTA[p, i] = i
            ones_t = cst[:, 256:257]       # ones column [128,1] f16
            stats_ps = ps_s.tile([1, 2 * C], f32, space="PSUM", tag="stats")
            ones_row = cp.tile([1, 128], f32)
            nc.vector.memset(ones_row[:], 1.0)
            a_it = cp.tile([128, n_chunks], i32)
            nc.sync.dma_start(out=a_it[:], in_=a_idx[:])
            b_it = cp.tile([128, NWIN * NWC], i32)
            nc.sync.dma_start(out=b_it[:], in_=b_idx[:])
            b_rt = cp.tile([128, NWIN * NWC], f16)
            nc.sync.dma_start(out=b_rt[:], in_=b_rowid[:])

            # ---------------- phase A ----------------
            cstage = None
            for k in range(K):
                for j in range(int(chunks_k[k])):
                    ch = int(k_chunk_base[k]) + j
                    g = sb.tile([128, C], f16, tag="g")
                    nc.gpsimd.indirect_dma_start(
                        out=g[:], out_offset=None, in_=feats[:],
                        in_offset=bass.IndirectOffsetOnAxis(
                            ap=a_it[:, ch:ch + 1], axis=0),
                    )
                    gt_ps = ps_t.tile([C, 128], f16, space="PSUM", tag="gtp")
                    nc.tensor.transpose(out=gt_ps[:], in_=g[:], identity=ident)
                    gt = sb.tile([C, 128], f16, tag="gt")
                    nc.scalar.copy(out=gt[:], in_=gt_ps[:])
                    c_ps = ps_c.tile([128, C], f32, space="PSUM", tag="cp")
                    nc.tensor.matmul(out=c_ps[:], lhsT=gt[:],
                                     rhs=w_t[:, k * C:(k + 1) * C],
                                     start=True, stop=True)
                    if ch % CB == 0:
                        cstage = sbig.tile([128, CB, C], f16, tag="cst")
                    nc.vector.tensor_copy(out=cstage[:, ch % CB, :], in_=c_ps[:])
                    if ch % CB == CB - 1:
                        c0 = (ch - (CB - 1)) * 128
                        nc.sync.dma_start(
                            out=cdram[c0:c0 + CB * 128, :].rearrange(
                                "(b p) c -> p b c", p=128),
                            in_=cstage[:])
            # (n_chunks is a multiple of CB only if chunks_k sums align; handle tail)
            rem = n_chunks % CB
            if rem:
                c0 = (n_chunks - rem) * 128
                nc.sync.dma_start(
                    out=cdram[c0:c0 + rem * 128, :].rearrange(
                        "(b p) c -> p b c", p=128),
                    in_=cstage[:, :rem, :])

            # ---------------- phase B ----------------
            for w in range(NWIN):
                win_ps = ps_w.tile([128, C], f32, space="PSUM", tag="win")
                for j in range(NWC):
                    col = w * NWC + j
                    cg = sb2.tile([128, C], f16, tag="cg")
                    nc.gpsimd.indirect_dma_start(
                        out=cg[:], out_offset=None, in_=cdram[:],
                        in_offset=bass.IndirectOffsetOnAxis(
                            ap=b_it[:, col:col + 1], axis=0),
                    )
                    selt = sb2.tile([128, 128], f16, tag="selt")
                    nc.vector.tensor_tensor(
                        out=selt[:],
                        in0=b_rt[:, col:col + 1].to_broadcast([128, 128]),
                        in1=iota,
                        op=mybir.AluOpType.is_equal,
                    )
                    nc.tensor.matmul(out=win_ps[:], lhsT=selt[:], rhs=cg[:],
                                     start=(j == 0), stop=(j == NWC - 1))
                win_sb = sb2.tile([128, C], f32, tag="winsb")
                nc.vector.tensor_copy(out=win_sb[:], in_=win_ps[:])
                nc.sync.dma_start(out=outp[w * 128:(w + 1) * 128, :], in_=win_sb[:])
                # stats
                win_h = sb2.tile([128, C], f16, tag="winh")
                nc.scalar.copy(out=win_h[:], in_=win_ps[:])
                sq_h = sb2.tile([128, C], f16, tag="sqh")
                nc.vector.tensor_mul(out=sq_h[:], in0=win_h[:], in1=win_h[:])
                nc.tensor.matmul(out=stats_ps[:, 0:C], lhsT=ones_t, rhs=win_h[:],
                                 start=(w == 0), stop=(w == NWIN - 1),
                                 skip_group_check=True)
                nc.tensor.matmul(out=stats_ps[:, C:2 * C], lhsT=ones_t, rhs=sq_h[:],
                                 start=(w == 0), stop=(w == NWIN - 1),
                                 skip_group_check=True)

            # stats -> allreduce
            st_sb = sb2.tile([1, 2 * C], f32)
            nc.vector.tensor_copy(out=st_sb[:], in_=stats_ps[:, :])
            nc.sync.dma_start(out=cc_in[:], in_=st_sb[:])
            nc.gpsimd.collective_compute(
                "AllReduce", mybir.AluOpType.add,
                replica_groups=[list(range(NCORES))],
                ins=[cc_in[:]], outs=[cc_out[:]],
            )
            st2 = sb2.tile([1, 2 * C], f32)
            nc.sync.dma_start(out=st2[:], in_=cc_out[:])
            gb_t = sb2.tile([1, 2 * C], f32)
            nc.sync.dma_start(out=gb_t[:], in_=gb[:])

            # scale = gamma * rsqrt(var+eps); bias = beta - mean*scale  (on [1, C])
            mean = sb2.tile([1, C], f32)
            nc.scalar.mul(out=mean[:], in_=st2[:, 0:C], mul=1.0 / N_OUT)
            ex2 = sb2.tile([1, C], f32)
            nc.scalar.mul(out=ex2[:], in_=st2[:, C:2 * C], mul=1.0 / N_OUT)
            m2 = sb2.tile([1, C], f32)
            nc.vector.tensor_mul(out=m2[:], in0=mean[:], in1=mean[:])
            var = sb2.tile([1, C], f32)
            nc.vector.tensor_sub(out=var[:], in0=ex2[:], in1=m2[:])
            eps_t = sb2.tile([1, 1], f32)
            nc.vector.memset(eps_t[:], BN_EPS)
            std = sb2.tile([1, C], f32)
            nc.scalar.activation(out=std[:], in_=var[:],
                                 func=mybir.ActivationFunctionType.Sqrt,
                                 bias=eps_t[:])
            rstd = sb2.tile([1, C], f32)
            nc.vector.reciprocal(out=rstd[:], in_=std[:])
            scale = sb2.tile([1, C], f32)
            nc.vector.tensor_mul(out=scale[:], in0=gb_t[:, 0:C], in1=rstd[:])
            nbias = sb2.tile([1, C], f32)
            nc.vector.tensor_mul(out=nbias[:], in0=mean[:], in1=scale[:])
            bias = sb2.tile([1, C], f32)
            nc.vector.tensor_sub(out=bias[:], in0=gb_t[:, C:2 * C], in1=nbias[:])

            # broadcast scale/bias to [128, C] via outer product with ones col
            sc_ps = ps_s.tile([128, 2 * C], f32, space="PSUM", tag="scps")
            nc.tensor.matmul(out=sc_ps[:, 0:C], lhsT=ones_row[:], rhs=scale[:],
                             start=True, stop=True, skip_group_check=True)
            nc.tensor.matmul(out=sc_ps[:, C:2 * C], lhsT=ones_row[:], rhs=bias[:],
                             start=True, stop=True, skip_group_check=True)
            sc_t = cp.tile([128, 2 * C], f32)
            nc.vector.tensor_copy(out=sc_t[:], in_=sc_ps[:])

            # ---------------- phase C: normalize + relu ----------------
            NB = 8
            for s in range(0, NWIN, NB):
                nb = min(NB, NWIN - s)
                o_t = sbig.tile([128, NB, C], f32, tag="ot")
                nc.sync.dma_start(
                    out=o_t[:, :nb, :],
                    in_=outp[s * 128:(s + nb) * 128, :].rearrange(
                        "(b p) c -> p b c", p=128))
                for b in range(nb):
                    nc.vector.tensor_mul(out=o_t[:, b, :], in0=o_t[:, b, :],
                                         in1=sc_t[:, 0:C])
                    nc.vector.tensor_add(out=o_t[:, b, :], in0=o_t[:, b, :],
                                         in1=sc_t[:, C:2 * C])
                y_t = sbig.tile([128, NB, C], f32, tag="yt")
                nc.scalar.activation(out=y_t[:, :nb, :], in_=o_t[:, :nb, :],
                                     func=mybir.ActivationFunctionType.Relu)
                nc.sync.dma_start(
                    out=y[s * 128:(s + nb) * 128, :].rearrange(
                        "(b p) c -> p b c", p=128),
                    in_=y_t[:, :nb, :])
    nc.compile()
    return nc


def kernel(**inputs):
    feats = np.asarray(inputs["feats"], dtype=np.float32)
    in_idx = np.asarray(inputs["in_idx"])
    out_idx = np.asarray(inputs["out_idx"])
    weight = np.asarray(inputs["weight"], dtype=np.float32)
    gamma = np.asarray(inputs["gamma"], dtype=np.float32)
    beta = np.asarray(inputs["beta"], dtype=np.float32)

    from concourse.bass_utils import run_bass_kernel_spmd

    prep = _host_prep(in_idx, out_idx)
    nc = _build(prep)

    feats_dev = np.zeros((N_IN + 1, C), dtype=np.float16)
    feats_dev[:N_IN] = feats.astype(np.float16)
    wdev = np.ascontiguousarray(
        weight.astype(np.float16).transpose(1, 0, 2).reshape(C, K * C))
    consts = np.zeros((128, 384), dtype=np.float16)
    consts[:, 0:128] = np.eye(128, dtype=np.float16)
    consts[:, 128:256] = np.arange(128, dtype=np.float16)[None, :]
    consts[:, 256] = 1.0
    gb = np.concatenate([gamma, beta]).astype(np.float32)[None, :]

    in_maps = []
    for c in range(NCORES):
        in_maps.append({
            "feats": feats_dev, "wmat": wdev, "consts": consts, "gb": gb,
            "a_idx": prep["A_idx"][c], "b_idx": prep["B_idx"][c],
            "b_rowid": prep["B_rowid"][c],
        })

    trace = bool(os.environ.get("BASS_KERNEL_TRACE"))
    if trace:
        try:
            _install_trace_shim()
        except Exception as e:
            print(f"trace shim unavailable ({e}); running untraced", file=sys.stderr)
            trace = False
    res = run_bass_kernel_spmd(nc, in_maps, core_ids=list(range(NCORES)),
                               trace=trace)
    if trace:
        _EXEC_TIME_NS[0] = res.exec_time_ns
    y = np.concatenate([res.results[c]["y"][:R_CORE] for c in range(NCORES)],
                       axis=0)
    return y.astype(np.float32)


def _install_trace_shim():
    """Register the NTFF profile hook (missing antenv.axon_hooks on this image)
    and neuter the S3 artifact upload so trace=True works under axon."""
    import types
    if "antenv.axon_hooks" not in sys.modules:
        mod = types.ModuleType("antenv.axon_hooks")
        mod._hook = None
        mod.set_axon_ntff_profile_hook = lambda h: setattr(mod, "_hook", h)
        mod.get_axon_ntff_profile_hook = lambda: mod._hook
        sys.modules["antenv.axon_hooks"] = mod
        sys.path.insert(0, "/root/.axon_site/trn_agent_boot")
        from trn_boot import _ntff_profile_via_ctypes
        mod._hook = _ntff_profile_via_ctypes("/opt/axon/libaxon_pjrt.so")
    import concourse.bass_utils as bu
    bu.upload_artifacts = lambda tmpdir: f"file://{tmpdir}"

